# revision 11
# baseline (speedup 1.0000x reference)
"""Haar wavelet transform (low, high) on Trainium2, 8-core data parallel.

Input  x: (8, 64, 512, 512) f32
Output (low, high): each (8, 64, 256, 256) f32
  For 2x2 blocks [[a,b],[c,d]]:
    low  = 0.5*(a+b+c+d)
    high = lh+hl+hh = 2*d - low

Sharding: batch dim -> 1 batch element per core (no cross-core comms).

The kernel is DMA-bound (16 SDMA engines at their ~27GB/s per-engine
ceiling), so all device traffic runs in bf16: the host pre-scales x by
0.5 and casts to bf16 (rel-err ~3e-3, tolerance 2e-2), halving HBM
bytes from 100.7MB to 48MB per core. With x' = x/2:
    low  = a'+b'+c'+d'
    high = 4*d' - low
The host also de-interleaves even/odd COLUMNS into row halves
(row = [even cols (256) | odd cols (256)]), so DVE ops are unit-stride
bf16 and auto-select the 2x perf mode; low/high go out row-interleaved
in ONE dram tensor.

Device pipeline decouples load tiling from compute tiling:
 - loads: 8 x 4MB tiles (4096 rows; 32KB per-partition descriptors --
   big descriptors are needed to stay at the DMA roofline), 4-slot ring
   on the SP HWDGE ring.
 - compute/stores: 16 half-tiles (2048 rows): DVE 5.5us per half-tile
   stays under the ~7us/half-tile DMA pace, and the end-of-kernel tail
   (last compute + last store after the final load) is halved vs
   full-tile compute. Stores (1MB, 8KB descriptors) on the ACT ring.
DVE per half-tile (all unit-stride bf16):
  s    = even_rows + odd_rows          -> per row [a+c | b+d]   (2x)
  low  = s[:, :256] + s[:, 256:]                                (2x)
  high = (d' * 4) - low                (scalar_tensor_tensor, 1x)
Only the last op carries then_inc (an EVENT_SEMAPHORE wait issued
right after an inc-carrying op stalls ~3.5us until the completion
retires), and the next half-tile's waits are hoisted right after op1.
"""

import sys

import numpy as np

for _p in ("/opt/trn_rl_repo",):
    if _p not in sys.path:
        sys.path.insert(0, _p)

# per-core problem geometry (hardcoded; one batch element per core)
_B = 8
_C, _H, _W = 64, 512, 512
_P = 128          # SBUF partitions
_RL = 32          # input rows per partition per LOAD tile
_RC = 16          # input rows per partition per COMPUTE half-tile
_ROWS = _C * _H   # 32768 input rows per core
_NL = _ROWS // (_P * _RL)   # 8 load tiles
_NC_ = _ROWS // (_P * _RC)  # 16 compute half-tiles
_OW = _W // 2
_OROWS = _ROWS // 2
_NBUF_IN = 4      # tin ring depth (load tiles)
_NBUF_OUT = 4     # out ring depth (half-tiles)

_prog_cache = {}


def _build_program():
    if "nc" in _prog_cache:
        return _prog_cache["nc"]
    import concourse.bass as bass
    from concourse import mybir

    bf16 = mybir.dt.bfloat16
    nc = bass.Bass()
    x = nc.declare_dram_parameter("x", [_ROWS, _W], bf16, isOutput=False)
    # low/high row-interleaved: out[r, 0, :] = low row r, out[r, 1, :] = high
    out = nc.declare_dram_parameter("out", [_OROWS, 2, _OW], bf16, isOutput=True)

    import contextlib

    with contextlib.ExitStack() as ctx:
        tin = [
            ctx.enter_context(
                nc.sbuf_tensor(f"tin{k}", [_P, _RL * _W], bf16)
            )
            for k in range(_NBUF_IN)
        ]
        s = ctx.enter_context(
            nc.sbuf_tensor("s", [_P, (_RC // 2) * _W], bf16)
        )
        ob = [
            ctx.enter_context(
                nc.sbuf_tensor(f"ob{k}", [_P, (_RC // 2) * 2 * _OW], bf16)
            )
            for k in range(_NBUF_OUT)
        ]
        # Per-ring-slot DMA sems: a slot's next DMA only dispatches after
        # the previous one was consumed, so "slot sem >= 16*count" exactly
        # means "all of this slot's DMAs landed on every SDMA engine".
        load_sem = [
            ctx.enter_context(nc.semaphore(f"load_sem{k}"))
            for k in range(_NBUF_IN)
        ]
        st_out = [
            ctx.enter_context(nc.semaphore(f"st_out{k}"))
            for k in range(_NBUF_OUT)
        ]
        dve_done = ctx.enter_context(nc.semaphore("dve_done"))
        block = ctx.enter_context(nc.Block())

        def in_src(j):
            nr = _RL * _P
            return x[j * nr : (j + 1) * nr, :].rearrange(
                "(p r) w -> p (r w)", p=_P
            )

        def out_dst(k):
            # partition p of load tile j holds input rows j*4096+p*32..+32,
            # so compute half h of that tile produces output rows
            # j*2048 + p*16 + h*8 .. +8 — select the h subgroup per partition
            j, h = k // 2, k % 2
            orows = _RL * _P // 2
            v = out[j * orows : (j + 1) * orows, :, :].rearrange(
                "(p q r) t j -> p q (r t j)", p=_P, q=2
            )
            return v[:, h, :]

        @block.sync
        def _(sync):
            # loads on the SP HWDGE ring
            for j in range(min(_NBUF_IN, _NL)):
                sync.dma_start(tin[j][:], in_src(j)).then_inc(
                    load_sem[j % _NBUF_IN], 16
                )
            for j in range(_NL - _NBUF_IN):
                # tin slot is free once both half-tiles of load j were
                # fully consumed (their STT high ops retired)
                sync.wait_ge(dve_done, 2 * (j + 1))
                jj = j + _NBUF_IN
                sync.dma_start(tin[jj % _NBUF_IN][:], in_src(jj)).then_inc(
                    load_sem[jj % _NBUF_IN], 16
                )

        @block.vector
        def _(vector):
            def tile_waits(k):
                j = k // 2
                vector.wait_ge(load_sem[j % _NBUF_IN], 16 * (j // _NBUF_IN + 1))
                if k >= _NBUF_OUT:
                    # out slot reuse: store of half-tile k-NBUF_OUT done
                    vector.wait_ge(st_out[k % _NBUF_OUT], 16 * (k // _NBUF_OUT))

            tile_waits(0)
            for k in range(_NC_):
                tb = tin[(k // 2) % _NBUF_IN]
                base = (k % 2) * _RC * _W
                t3in = tb[:, base : base + _RC * _W].rearrange(
                    "p (r w) -> p r w", w=_W
                )
                ev = t3in[:, 0::2, :]
                od = t3in[:, 1::2, :]
                # odd rows, odd cols = right half of od (host de-interleave)
                d = t3in[:, 1::2, _OW:]
                s3 = s[:].rearrange("p (k w) -> p k w", w=_W)
                o3 = ob[k % _NBUF_OUT][:].rearrange("p (k w) -> p k w", w=2 * _OW)
                lo3 = o3[:, :, :_OW]
                hi3 = o3[:, :, _OW:]
                nc.vector.tensor_add(s3, ev, od)
                if k + 1 < _NC_:
                    tile_waits(k + 1)
                nc.vector.tensor_add(lo3, s3[:, :, :_OW], s3[:, :, _OW:])
                nc.vector.scalar_tensor_tensor(
                    hi3, d, 4.0, lo3,
                    mybir.AluOpType.mult, mybir.AluOpType.subtract,
                ).then_inc(dve_done, 1)

        @block.scalar
        def _(scalar):
            # stores on the ACT HWDGE ring
            for k in range(_NC_):
                scalar.wait_ge(dve_done, k + 1)
                scalar.dma_start(out_dst(k), ob[k % _NBUF_OUT][:]).then_inc(
                    st_out[k % _NBUF_OUT], 16
                )
            # final: all stores landed
            for m in range(_NBUF_OUT):
                nslot = len([k for k in range(_NC_) if k % _NBUF_OUT == m])
                scalar.wait_ge(st_out[m], 16 * nslot)

    _prog_cache["nc"] = nc
    return nc


def _run(x: np.ndarray, trace: bool = False):
    import ml_dtypes

    from concourse.bass_utils import run_bass_kernel_spmd

    nc = _build_program()
    xs = np.asarray(x).reshape(_B, _ROWS, _W)
    assert xs.shape == (_B, _ROWS, _W), xs.shape
    # fold the Haar 0.5 into a host-side pre-scale, cast to bf16, and
    # de-interleave even/odd columns into row halves so the device ops
    # are all unit-stride
    half = np.float32(0.5)
    xh = np.empty((_B, _ROWS, _W), dtype=ml_dtypes.bfloat16)
    xh[:, :, : _OW] = xs[:, :, 0::2] * half
    xh[:, :, _OW :] = xs[:, :, 1::2] * half
    in_maps = [{"x": xh[b]} for b in range(_B)]
    out = run_bass_kernel_spmd(nc, in_maps, list(range(_B)), trace=trace)
    lows, highs = [], []
    for b in range(_B):
        ob = np.asarray(out.results[b]["out"], dtype=np.float32).reshape(
            _C, _H // 2, 2, _W // 2
        )
        lows.append(ob[:, :, 0, :])
        highs.append(ob[:, :, 1, :])
    return (np.stack(lows), np.stack(highs)), out


def kernel(x: np.ndarray):
    (low, high), _ = _run(x, trace=False)
    return low, high


# revision 12
# speedup vs baseline: 1.1563x; 1.1563x over previous
"""Haar wavelet transform (low, high) on Trainium2, 8-core data parallel.

Input  x: (8, 64, 512, 512) f32
Output (low, high): each (8, 64, 256, 256) f32
  For 2x2 blocks [[a,b],[c,d]]:
    low  = 0.5*(a+b+c+d)
    high = lh+hl+hh = 2*d - low

Sharding: batch dim -> 1 batch element per core (no cross-core comms).

The kernel is DMA-bound (16 SDMA engines at their ~27GB/s per-engine
ceiling), so all device traffic runs in bf16: the host pre-scales x by
0.5 and casts to bf16 (rel-err ~3e-3, tolerance 2e-2), halving HBM
bytes from 100.7MB to 48MB per core. With x' = x/2:
    low  = a'+b'+c'+d'
    high = 4*d' - low
The host also de-interleaves even/odd COLUMNS into row halves
(row = [even cols (256) | odd cols (256)]), so DVE ops are unit-stride
bf16 and auto-select the 2x perf mode; low/high go out row-interleaved
in ONE dram tensor.

Device pipeline decouples load tiling from compute tiling:
 - loads: 8 x 4MB tiles (4096 rows; 32KB per-partition descriptors --
   big descriptors are needed to stay at the DMA roofline), 4-slot ring
   on the SP HWDGE ring.
 - compute/stores: 16 half-tiles (2048 rows): DVE 5.5us per half-tile
   stays under the ~7us/half-tile DMA pace, and the end-of-kernel tail
   (last compute + last store after the final load) is halved vs
   full-tile compute. Stores (1MB, 8KB descriptors) on the ACT ring.
DVE per half-tile (all unit-stride bf16):
  s    = even_rows + odd_rows          -> per row [a+c | b+d]   (2x)
  low  = s[:, :256] + s[:, 256:]                                (2x)
  high = (d' * 4) - low                (scalar_tensor_tensor, 1x)
Only the last op carries then_inc (an EVENT_SEMAPHORE wait issued
right after an inc-carrying op stalls ~3.5us until the completion
retires), and the next half-tile's waits are hoisted right after op1.
"""

import sys

import numpy as np

for _p in ("/opt/trn_rl_repo",):
    if _p not in sys.path:
        sys.path.insert(0, _p)

# per-core problem geometry (hardcoded; one batch element per core)
_B = 8
_C, _H, _W = 64, 512, 512
_P = 128          # SBUF partitions
_RL = 32          # input rows per partition per LOAD tile
_RC = 16          # input rows per partition per COMPUTE half-tile
_ROWS = _C * _H   # 32768 input rows per core
_NL = _ROWS // (_P * _RL)   # 8 load tiles
_NC_ = _ROWS // (_P * _RC)  # 16 compute half-tiles
_OW = _W // 2
_OROWS = _ROWS // 2
_NBUF_IN = 4      # tin ring depth (load tiles)
_NBUF_OUT = 6     # out ring depth (half-tiles; deep enough that store
                  # completion latency (~13us behind dispatch when queued
                  # after loads) never blocks DVE's ob-slot reuse wait

_prog_cache = {}


def _build_program():
    if "nc" in _prog_cache:
        return _prog_cache["nc"]
    import concourse.bass as bass
    from concourse import mybir

    bf16 = mybir.dt.bfloat16
    nc = bass.Bass()
    x = nc.declare_dram_parameter("x", [_ROWS, _W], bf16, isOutput=False)
    # low/high row-interleaved: out[r, 0, :] = low row r, out[r, 1, :] = high
    out = nc.declare_dram_parameter("out", [_OROWS, 2, _OW], bf16, isOutput=True)

    import contextlib

    with contextlib.ExitStack() as ctx:
        tin = [
            ctx.enter_context(
                nc.sbuf_tensor(f"tin{k}", [_P, _RL * _W], bf16)
            )
            for k in range(_NBUF_IN)
        ]
        s = ctx.enter_context(
            nc.sbuf_tensor("s", [_P, (_RC // 2) * _W], bf16)
        )
        ob = [
            ctx.enter_context(
                nc.sbuf_tensor(f"ob{k}", [_P, (_RC // 2) * 2 * _OW], bf16)
            )
            for k in range(_NBUF_OUT)
        ]
        # Per-ring-slot DMA sems: a slot's next DMA only dispatches after
        # the previous one was consumed, so "slot sem >= 16*count" exactly
        # means "all of this slot's DMAs landed on every SDMA engine".
        load_sem = [
            ctx.enter_context(nc.semaphore(f"load_sem{k}"))
            for k in range(_NBUF_IN)
        ]
        st_out = [
            ctx.enter_context(nc.semaphore(f"st_out{k}"))
            for k in range(_NBUF_OUT)
        ]
        dve_done = ctx.enter_context(nc.semaphore("dve_done"))
        block = ctx.enter_context(nc.Block())

        def in_src(j):
            nr = _RL * _P
            return x[j * nr : (j + 1) * nr, :].rearrange(
                "(p r) w -> p (r w)", p=_P
            )

        def out_dst(k):
            # partition p of load tile j holds input rows j*4096+p*32..+32,
            # so compute half h of that tile produces output rows
            # j*2048 + p*16 + h*8 .. +8 — select the h subgroup per partition
            j, h = k // 2, k % 2
            orows = _RL * _P // 2
            v = out[j * orows : (j + 1) * orows, :, :].rearrange(
                "(p q r) t j -> p q (r t j)", p=_P, q=2
            )
            return v[:, h, :]

        @block.sync
        def _(sync):
            # loads on the SP HWDGE ring
            for j in range(min(_NBUF_IN, _NL)):
                sync.dma_start(tin[j][:], in_src(j)).then_inc(
                    load_sem[j % _NBUF_IN], 16
                )
            for j in range(_NL - _NBUF_IN):
                # tin slot is free once both half-tiles of load j were
                # fully consumed (their STT high ops retired)
                sync.wait_ge(dve_done, 2 * (j + 1))
                jj = j + _NBUF_IN
                sync.dma_start(tin[jj % _NBUF_IN][:], in_src(jj)).then_inc(
                    load_sem[jj % _NBUF_IN], 16
                )

        @block.vector
        def _(vector):
            def tile_waits(k):
                j = k // 2
                vector.wait_ge(load_sem[j % _NBUF_IN], 16 * (j // _NBUF_IN + 1))
                if k >= _NBUF_OUT:
                    # out slot reuse: store of half-tile k-NBUF_OUT done
                    vector.wait_ge(st_out[k % _NBUF_OUT], 16 * (k // _NBUF_OUT))

            tile_waits(0)
            for k in range(_NC_):
                tb = tin[(k // 2) % _NBUF_IN]
                base = (k % 2) * _RC * _W
                t3in = tb[:, base : base + _RC * _W].rearrange(
                    "p (r w) -> p r w", w=_W
                )
                ev = t3in[:, 0::2, :]
                od = t3in[:, 1::2, :]
                # odd rows, odd cols = right half of od (host de-interleave)
                d = t3in[:, 1::2, _OW:]
                s3 = s[:].rearrange("p (k w) -> p k w", w=_W)
                o3 = ob[k % _NBUF_OUT][:].rearrange("p (k w) -> p k w", w=2 * _OW)
                lo3 = o3[:, :, :_OW]
                hi3 = o3[:, :, _OW:]
                nc.vector.tensor_add(s3, ev, od)
                if k + 1 < _NC_:
                    tile_waits(k + 1)
                nc.vector.tensor_add(lo3, s3[:, :, :_OW], s3[:, :, _OW:])
                nc.vector.scalar_tensor_tensor(
                    hi3, d, 4.0, lo3,
                    mybir.AluOpType.mult, mybir.AluOpType.subtract,
                ).then_inc(dve_done, 1)

        @block.scalar
        def _(scalar):
            # stores on the ACT HWDGE ring
            for k in range(_NC_):
                scalar.wait_ge(dve_done, k + 1)
                scalar.dma_start(out_dst(k), ob[k % _NBUF_OUT][:]).then_inc(
                    st_out[k % _NBUF_OUT], 16
                )
            # final: all stores landed
            for m in range(_NBUF_OUT):
                nslot = len([k for k in range(_NC_) if k % _NBUF_OUT == m])
                scalar.wait_ge(st_out[m], 16 * nslot)

    _prog_cache["nc"] = nc
    return nc


def _run(x: np.ndarray, trace: bool = False):
    import ml_dtypes

    from concourse.bass_utils import run_bass_kernel_spmd

    nc = _build_program()
    xs = np.asarray(x).reshape(_B, _ROWS, _W)
    assert xs.shape == (_B, _ROWS, _W), xs.shape
    # fold the Haar 0.5 into a host-side pre-scale, cast to bf16, and
    # de-interleave even/odd columns into row halves so the device ops
    # are all unit-stride
    half = np.float32(0.5)
    xh = np.empty((_B, _ROWS, _W), dtype=ml_dtypes.bfloat16)
    xh[:, :, : _OW] = xs[:, :, 0::2] * half
    xh[:, :, _OW :] = xs[:, :, 1::2] * half
    in_maps = [{"x": xh[b]} for b in range(_B)]
    out = run_bass_kernel_spmd(nc, in_maps, list(range(_B)), trace=trace)
    lows, highs = [], []
    for b in range(_B):
        ob = np.asarray(out.results[b]["out"], dtype=np.float32).reshape(
            _C, _H // 2, 2, _W // 2
        )
        lows.append(ob[:, :, 0, :])
        highs.append(ob[:, :, 1, :])
    return (np.stack(lows), np.stack(highs)), out


def kernel(x: np.ndarray):
    (low, high), _ = _run(x, trace=False)
    return low, high
